# revision 2
# baseline (speedup 1.0000x reference)
"""CRF decode (conv features -> emission scores -> Viterbi) on 8 TRN2 cores.

Data-parallel over the batch: each core gets B/8 = 4096 words. Per core:
  - conv+emission collapse to one (128 -> 26) linear map A = W @ C applied on
    the PE per 128-word tile (one PE transpose + one matmul per letter),
  - Viterbi forward DP on the DVE with words on partitions: per step a
    (128, 26, 26) candidate add, grouped max, first-argmax via is_ge * (26-i)
    + grouped max, then the emission add,
  - batched backtrack over all tiles via one-hot selects,
  - int32 convert + strided DMA out.
"""

import sys

if "/opt/trn_rl_repo" not in sys.path:
    sys.path.insert(0, "/opt/trn_rl_repo")

import numpy as np

import concourse.bacc as bacc
import concourse.mybir as mybir
import concourse.tile as tile
from concourse import bass_utils

F32 = mybir.dt.float32
AX = mybir.AxisListType
OP = mybir.AluOpType

B = 32768
M = 14
H, WD = 16, 8
F = 128
L = 26
KS = 5
NCORES = 8
BC = B // NCORES          # words per core
NT = BC // 128            # 128-word tiles per core (32)


def _conv_matrix(K: np.ndarray) -> np.ndarray:
    """C[o, i] such that conv_SAME(x.reshape(H,WD)) flattened == C @ x."""
    K2 = K.reshape(KS, KS).astype(np.float64)
    C = np.zeros((F, F), dtype=np.float64)
    for r in range(H):
        for c in range(WD):
            o = r * WD + c
            for dy in range(KS):
                for dx in range(KS):
                    rr = r + dy - KS // 2
                    cc = c + dx - KS // 2
                    if 0 <= rr < H and 0 <= cc < WD:
                        C[o, rr * WD + cc] = K2[dy, dx]
    return C


def _consts(K, b, W, T):
    """Host-side constant tensors (fp64 math, one final fp32 round)."""
    C = _conv_matrix(K)
    A = W.astype(np.float64) @ C                       # (L, F)
    c0 = float(b[0]) * W.astype(np.float64).sum(axis=1)  # (L,)
    Tp = T.astype(np.float64) + c0[None, :]            # T'[i,j] = T[i,j]+c0[j]
    AT = np.ascontiguousarray(A.T).astype(np.float32)  # (F, L)
    TTK = np.broadcast_to(
        np.ascontiguousarray(Tp.T).astype(np.float32)[None], (128, L, L)
    ).copy()                                           # TTK[p, j, i] = T'[i, j]
    C0B = np.broadcast_to(c0.astype(np.float32)[None], (128, L)).copy()
    IR = np.broadcast_to(
        (L - np.arange(L)).astype(np.float32)[None], (128, L)
    ).copy()                                           # 26 - i
    IOTA = np.broadcast_to(
        np.arange(L, dtype=np.float32)[None], (128, L)
    ).copy()
    IDN = np.eye(128, dtype=np.float32)
    return AT, TTK, C0B, IR, IOTA, IDN


def build_module():
    nc = bacc.Bacc("TRN2", target_bir_lowering=False, debug=False,
                   num_devices=NCORES)
    xs = nc.dram_tensor("XS", [BC, M, F], F32, kind="ExternalInput")
    at_d = nc.dram_tensor("AT", [F, L], F32, kind="ExternalInput")
    ttk_d = nc.dram_tensor("TTK", [128, L, L], F32, kind="ExternalInput")
    c0_d = nc.dram_tensor("C0B", [128, L], F32, kind="ExternalInput")
    ir_d = nc.dram_tensor("IR", [128, L], F32, kind="ExternalInput")
    io_d = nc.dram_tensor("IOTA", [128, L], F32, kind="ExternalInput")
    id_d = nc.dram_tensor("IDN", [128, 128], F32, kind="ExternalInput")
    out_d = nc.dram_tensor("OUT", [BC, M], mybir.dt.int32,
                           kind="ExternalOutput")

    with tile.TileContext(nc) as tc:
        with (
            tc.tile_pool(name="const", bufs=1) as cpool,
            tc.tile_pool(name="pers", bufs=1) as ppool,
            tc.tile_pool(name="work", bufs=3) as wpool,
            tc.tile_pool(name="dp", bufs=2) as dpool,
            tc.tile_pool(name="psum", bufs=2, space="PSUM") as psA,
            tc.tile_pool(name="psum2", bufs=2, space="PSUM") as psB,
        ):
            at = cpool.tile([F, L], F32)
            ttk = cpool.tile([128, L, L], F32)
            c0b = cpool.tile([128, L], F32)
            ir = cpool.tile([128, L], F32)
            iota = cpool.tile([128, L], F32)
            idn = cpool.tile([128, 128], F32)
            nc.sync.dma_start(at[:], at_d.ap())
            nc.sync.dma_start(ttk[:], ttk_d.ap())
            nc.sync.dma_start(c0b[:], c0_d.ap())
            nc.sync.dma_start(ir[:], ir_d.ap())
            nc.sync.dma_start(iota[:], io_d.ap())
            nc.sync.dma_start(idn[:], id_d.ap())

            bp = ppool.tile([128, NT, M - 1, L], F32)   # 26 - argmax, per step
            vall = ppool.tile([128, NT, L], F32)        # final v per tile
            path = ppool.tile([128, NT, M], F32)
            ir_b = ir[:].unsqueeze(1).broadcast_to((128, L, L))

            xs_t = xs.ap().rearrange("(n p) m f -> n p (m f)", p=128)

            for wt in range(NT):
                xt = wpool.tile([128, M * F], F32, tag="xt")
                nc.sync.dma_start(xt[:], xs_t[wt])
                sc = wpool.tile([128, M, L], F32, tag="sc")
                for m in range(M):
                    xT = psA.tile([128, 128], F32, tag="xT")
                    nc.tensor.transpose(xT[:], xt[:, m * F:(m + 1) * F], idn[:])
                    xTs = wpool.tile([128, 128], F32, tag="xTs")
                    nc.scalar.activation(
                        xTs[:], xT[:], mybir.ActivationFunctionType.Copy)
                    scp = psB.tile([128, L], F32, tag="scp")
                    nc.tensor.matmul(scp[:], xTs[:], at[:])
                    nc.scalar.activation(
                        sc[:, m, :], scp[:], mybir.ActivationFunctionType.Copy)

                v = vall[:, wt, :]
                nc.vector.tensor_tensor(v, sc[:, 0, :], c0b[:], op=OP.add)
                v_b = v.unsqueeze(1).broadcast_to((128, L, L))
                for t in range(1, M):
                    cv = dpool.tile([128, L, L], F32, tag="cv")
                    mx = dpool.tile([128, L], F32, tag="mx")
                    nc.vector.tensor_tensor(cv[:], v_b, ttk[:], op=OP.add)
                    nc.vector.tensor_reduce(mx[:], cv[:], axis=AX.X, op=OP.max)
                    m_b = mx[:].unsqueeze(2).broadcast_to((128, L, L))
                    nc.vector.tensor_tensor(cv[:], cv[:], m_b, op=OP.is_ge)
                    nc.vector.tensor_tensor(cv[:], cv[:], ir_b, op=OP.mult)
                    nc.vector.tensor_reduce(
                        bp[:, wt, t - 1, :], cv[:], axis=AX.X, op=OP.max)
                    nc.vector.tensor_tensor(v, mx[:], sc[:, t, :], op=OP.add)

            # batched backtrack over all tiles
            ew = ppool.tile([128, NT, L], F32)
            rw = ppool.tile([128, NT], F32)
            ir_bt = ir[:].unsqueeze(1).broadcast_to((128, NT, L))
            io_bt = iota[:].unsqueeze(1).broadcast_to((128, NT, L))

            nc.vector.tensor_reduce(rw[:], vall[:], axis=AX.X, op=OP.max)
            nc.vector.tensor_tensor(
                ew[:], vall[:], rw[:].unsqueeze(2).broadcast_to((128, NT, L)),
                op=OP.is_ge)
            nc.vector.tensor_tensor(ew[:], ew[:], ir_bt, op=OP.mult)
            nc.vector.tensor_reduce(rw[:], ew[:], axis=AX.X, op=OP.max)
            nc.vector.tensor_scalar(
                path[:, :, M - 1], rw[:], -1.0, float(L), op0=OP.mult,
                op1=OP.add)
            for t in range(M - 2, -1, -1):
                nxt = path[:, :, t + 1].unsqueeze(2).broadcast_to((128, NT, L))
                nc.vector.tensor_tensor(ew[:], io_bt, nxt, op=OP.is_equal)
                nc.vector.tensor_tensor(ew[:], ew[:], bp[:, :, t, :], op=OP.mult)
                nc.vector.tensor_reduce(rw[:], ew[:], axis=AX.X, op=OP.max)
                nc.vector.tensor_scalar(
                    path[:, :, t], rw[:], -1.0, float(L), op0=OP.mult,
                    op1=OP.add)

            pi = ppool.tile([128, NT, M], mybir.dt.int32)
            nc.vector.tensor_copy(pi[:], path[:])
            out_t = out_d.ap().rearrange("(n p) m -> p n m", p=128)
            nc.sync.dma_start(out_t, pi[:])

    nc.compile()
    return nc


_CACHE = {}


def _get_module():
    if "nc" not in _CACHE:
        _CACHE["nc"] = build_module()
    return _CACHE["nc"]


def make_in_maps(X, K, b, W, T):
    AT, TTK, C0B, IR, IOTA, IDN = _consts(K, b, W, T)
    consts = {"AT": AT, "TTK": TTK, "C0B": C0B, "IR": IR, "IOTA": IOTA,
              "IDN": IDN}
    X = np.ascontiguousarray(X, dtype=np.float32)
    return [dict(consts, XS=X[c * BC:(c + 1) * BC]) for c in range(NCORES)]


def kernel(X, K, b, W, T):
    nc = _get_module()
    in_maps = make_in_maps(X, K, b, W, T)
    res = bass_utils.run_bass_kernel_spmd(nc, in_maps,
                                          core_ids=list(range(NCORES)))
    out = np.concatenate([res.results[c]["OUT"] for c in range(NCORES)], axis=0)
    return out.reshape(B, M, 1).astype(np.int32)


# revision 5
# speedup vs baseline: 1.0099x; 1.0099x over previous
"""CRF decode (conv features -> emission scores -> Viterbi) on 8 TRN2 cores.

Data-parallel over the batch: each core gets B/8 = 4096 words. Per core:
  - conv+emission collapse to one (128 -> 26) linear map A = W @ C applied on
    the PE per 128-word tile (one PE transpose + one matmul per letter),
  - Viterbi forward DP on the DVE with words on partitions: per step a
    (128, 26, 26) candidate add, grouped max, first-argmax via is_ge * (26-i)
    + grouped max, then the emission add,
  - batched backtrack over all tiles via one-hot selects,
  - int32 convert + strided DMA out.
"""

import sys

if "/opt/trn_rl_repo" not in sys.path:
    sys.path.insert(0, "/opt/trn_rl_repo")

import numpy as np

import concourse.bacc as bacc
import concourse.mybir as mybir
import concourse.tile as tile
from concourse import bass_utils
from concourse import dve_ops
from concourse.dve_ops import DveOp
from concourse.dve_spec import Spec, Src0, Src1, C0, C1, Idx, SubIdx, lower
from concourse.dve_table_gen import dve_ver_for
from concourse.dve_uop import DveOpSpec


def _register_geq_revidx():
    """Fused (c >= m_bcast) * (26 - within_page_idx) as one DVE pass.

    For a [P, S, N] stream (S pages of N labels), out[p,s,n] is
    (in0 >= in1) * (C0 - (Idx - C1*SubIdx)); with C0=C1=N the factor is
    N - n, so a grouped max over n of the output yields N - first_argmax.
    """
    if "GEQ_REVIDX" in dve_ops._SUB_OPCODE_FOR_NAME:
        return dve_ops.CUSTOM_DVE_SPECS["GEQ_REVIDX"] and None

    def _ref(in0, in1, s0, s1, imm2):
        P = in0.shape[0]
        a = in0.reshape(P, -1).astype(np.float32)
        b = np.asarray(in1).reshape(P, -1).astype(np.float32)
        S = in0.shape[1] if in0.ndim == 3 else 1
        N = a.shape[1] // S
        idx = np.arange(S * N, dtype=np.float32)
        sub = np.repeat(np.arange(S, dtype=np.float32), N)
        w = (sub * s1 - idx) + s0
        return ((a >= b).astype(np.float32) * w[None, :]).reshape(in0.shape)

    spec = Spec(
        body=(Src0 >= Src1) * ((SubIdx * C1 - Idx) + C0),
        reference=_ref,
    )
    shas = {}
    opcode = max(dve_ops._SUB_OPCODE_FOR_NAME.values()) + 1
    for ver in ("v3", "v4"):
        s = DveOpSpec(name="GEQ_REVIDX", opcode=opcode,
                      uops=lower(spec, ver=ver), rd1_en=True)
        shas[ver] = s.sha(ver)
    op = DveOp("GEQ_REVIDX", spec, subdim=True, uops_sha=shas)
    dve_ops.OPS.append(op)
    dve_ops.CUSTOM_DVE_SPECS["GEQ_REVIDX"] = spec
    dve_ops._SUB_OPCODE_FOR_NAME["GEQ_REVIDX"] = opcode
    return op


GEQ_REVIDX = _register_geq_revidx() or dve_ops.OPS[-1]

F32 = mybir.dt.float32
AX = mybir.AxisListType
OP = mybir.AluOpType

B = 32768
M = 14
H, WD = 16, 8
F = 128
L = 26
KS = 5
NCORES = 8
BC = B // NCORES          # words per core
NT = BC // 128            # 128-word tiles per core (32)


def _conv_matrix(K: np.ndarray) -> np.ndarray:
    """C[o, i] such that conv_SAME(x.reshape(H,WD)) flattened == C @ x."""
    K2 = K.reshape(KS, KS).astype(np.float64)
    C = np.zeros((F, F), dtype=np.float64)
    for r in range(H):
        for c in range(WD):
            o = r * WD + c
            for dy in range(KS):
                for dx in range(KS):
                    rr = r + dy - KS // 2
                    cc = c + dx - KS // 2
                    if 0 <= rr < H and 0 <= cc < WD:
                        C[o, rr * WD + cc] = K2[dy, dx]
    return C


def _consts(K, b, W, T):
    """Host-side constant tensors (fp64 math, one final fp32 round)."""
    C = _conv_matrix(K)
    A = W.astype(np.float64) @ C                       # (L, F)
    c0 = float(b[0]) * W.astype(np.float64).sum(axis=1)  # (L,)
    Tp = T.astype(np.float64) + c0[None, :]            # T'[i,j] = T[i,j]+c0[j]
    AT = np.ascontiguousarray(A.T).astype(np.float32)  # (F, L)
    TTK = np.broadcast_to(
        np.ascontiguousarray(Tp.T).astype(np.float32)[None], (128, L, L)
    ).copy()                                           # TTK[p, j, i] = T'[i, j]
    C0B = np.broadcast_to(c0.astype(np.float32)[None], (128, L)).copy()
    IR = np.broadcast_to(
        (L - np.arange(L)).astype(np.float32)[None], (128, L)
    ).copy()                                           # 26 - i
    IOTA = np.broadcast_to(
        np.arange(L, dtype=np.float32)[None], (128, L)
    ).copy()
    IDN = np.eye(128, dtype=np.float32)
    return AT, TTK, C0B, IR, IOTA, IDN


def build_module():
    nc = bacc.Bacc("TRN2", target_bir_lowering=False, debug=False,
                   num_devices=NCORES)
    xs = nc.dram_tensor("XS", [BC, M, F], F32, kind="ExternalInput")
    at_d = nc.dram_tensor("AT", [F, L], F32, kind="ExternalInput")
    ttk_d = nc.dram_tensor("TTK", [128, L, L], F32, kind="ExternalInput")
    c0_d = nc.dram_tensor("C0B", [128, L], F32, kind="ExternalInput")
    ir_d = nc.dram_tensor("IR", [128, L], F32, kind="ExternalInput")
    io_d = nc.dram_tensor("IOTA", [128, L], F32, kind="ExternalInput")
    id_d = nc.dram_tensor("IDN", [128, 128], F32, kind="ExternalInput")
    out_d = nc.dram_tensor("OUT", [BC, M], mybir.dt.int32,
                           kind="ExternalOutput")

    with tile.TileContext(nc) as tc:
        with (
            tc.tile_pool(name="const", bufs=1) as cpool,
            tc.tile_pool(name="pers", bufs=1) as ppool,
            tc.tile_pool(name="work", bufs=3) as wpool,
            tc.tile_pool(name="dp", bufs=2) as dpool,
            tc.tile_pool(name="psum", bufs=2, space="PSUM") as psA,
            tc.tile_pool(name="psum2", bufs=2, space="PSUM") as psB,
        ):
            at = cpool.tile([F, L], F32)
            ttk = cpool.tile([128, L, L], F32)
            c0b = cpool.tile([128, L], F32)
            ir = cpool.tile([128, L], F32)
            iota = cpool.tile([128, L], F32)
            idn = cpool.tile([128, 128], F32)
            nc.sync.dma_start(at[:], at_d.ap())
            nc.sync.dma_start(ttk[:], ttk_d.ap())
            nc.sync.dma_start(c0b[:], c0_d.ap())
            nc.sync.dma_start(ir[:], ir_d.ap())
            nc.sync.dma_start(iota[:], io_d.ap())
            nc.sync.dma_start(idn[:], id_d.ap())

            bp = ppool.tile([128, NT, M - 1, L], F32)   # 26 - argmax, per step
            vall = ppool.tile([128, NT, L], F32)        # final v per tile
            path = ppool.tile([128, NT, M], F32)
            ir_b = ir[:].unsqueeze(1).broadcast_to((128, L, L))

            xs_t = xs.ap().rearrange("(n p) m f -> n p (m f)", p=128)

            for wt in range(NT):
                xt = wpool.tile([128, M * F], F32, tag="xt")
                nc.sync.dma_start(xt[:], xs_t[wt])
                sc = wpool.tile([128, M, L], F32, tag="sc")
                for m in range(M):
                    xT = psA.tile([128, 128], F32, tag="xT")
                    nc.tensor.transpose(xT[:], xt[:, m * F:(m + 1) * F], idn[:])
                    xTs = wpool.tile([128, 128], F32, tag="xTs")
                    nc.scalar.activation(
                        xTs[:], xT[:], mybir.ActivationFunctionType.Copy)
                    scp = psB.tile([128, L], F32, tag="scp")
                    nc.tensor.matmul(scp[:], xTs[:], at[:])
                    nc.scalar.activation(
                        sc[:, m, :], scp[:], mybir.ActivationFunctionType.Copy)

                v = vall[:, wt, :]
                nc.vector.tensor_tensor(v, sc[:, 0, :], c0b[:], op=OP.add)
                v_b = v.unsqueeze(1).broadcast_to((128, L, L))
                use_gp = False   # argmax pass on GPSIMD for 4/5 tiles
                for t in range(1, M):
                    cv = dpool.tile([128, L, L], F32, tag="cv")
                    mx = dpool.tile([128, L], F32, tag="mx")
                    nc.vector.tensor_tensor(cv[:], v_b, ttk[:], op=OP.add)
                    nc.vector.tensor_reduce(mx[:], cv[:], axis=AX.X, op=OP.max)
                    m_b = mx[:].unsqueeze(2).broadcast_to((128, L, L))
                    if use_gp:
                        nc.gpsimd.tensor_tensor(cv[:], cv[:], m_b, op=OP.is_ge)
                        nc.gpsimd.tensor_tensor(cv[:], cv[:], ir_b, op=OP.mult)
                    else:
                        nc.vector._custom_dve(
                            GEQ_REVIDX, out=cv[:], in0=cv[:], in1=m_b,
                            s0=float(L), s1=float(L))
                    nc.vector.tensor_reduce(
                        bp[:, wt, t - 1, :], cv[:], axis=AX.X, op=OP.max)
                    nc.vector.tensor_tensor(v, mx[:], sc[:, t, :], op=OP.add)

            # batched backtrack over all tiles
            ew = ppool.tile([128, NT, L], F32)
            rw = ppool.tile([128, NT], F32)
            ir_bt = ir[:].unsqueeze(1).broadcast_to((128, NT, L))
            io_bt = iota[:].unsqueeze(1).broadcast_to((128, NT, L))

            nc.vector.tensor_reduce(rw[:], vall[:], axis=AX.X, op=OP.max)
            nc.vector.tensor_tensor(
                ew[:], vall[:], rw[:].unsqueeze(2).broadcast_to((128, NT, L)),
                op=OP.is_ge)
            nc.vector.tensor_tensor(ew[:], ew[:], ir_bt, op=OP.mult)
            nc.vector.tensor_reduce(rw[:], ew[:], axis=AX.X, op=OP.max)
            nc.vector.tensor_scalar(
                path[:, :, M - 1], rw[:], -1.0, float(L), op0=OP.mult,
                op1=OP.add)
            for t in range(M - 2, -1, -1):
                nxt = path[:, :, t + 1].unsqueeze(2).broadcast_to((128, NT, L))
                nc.vector.tensor_tensor(ew[:], io_bt, nxt, op=OP.is_equal)
                nc.vector.tensor_tensor(ew[:], ew[:], bp[:, :, t, :], op=OP.mult)
                nc.vector.tensor_reduce(rw[:], ew[:], axis=AX.X, op=OP.max)
                nc.vector.tensor_scalar(
                    path[:, :, t], rw[:], -1.0, float(L), op0=OP.mult,
                    op1=OP.add)

            pi = ppool.tile([128, NT, M], mybir.dt.int32)
            nc.vector.tensor_copy(pi[:], path[:])
            out_t = out_d.ap().rearrange("(n p) m -> p n m", p=128)
            nc.sync.dma_start(out_t, pi[:])

    nc.compile()
    return nc


_CACHE = {}


def _get_module():
    if "nc" not in _CACHE:
        _CACHE["nc"] = build_module()
    return _CACHE["nc"]


def make_in_maps(X, K, b, W, T):
    AT, TTK, C0B, IR, IOTA, IDN = _consts(K, b, W, T)
    consts = {"AT": AT, "TTK": TTK, "C0B": C0B, "IR": IR, "IOTA": IOTA,
              "IDN": IDN}
    X = np.ascontiguousarray(X, dtype=np.float32)
    return [dict(consts, XS=X[c * BC:(c + 1) * BC]) for c in range(NCORES)]


def kernel(X, K, b, W, T):
    nc = _get_module()
    in_maps = make_in_maps(X, K, b, W, T)
    res = bass_utils.run_bass_kernel_spmd(nc, in_maps,
                                          core_ids=list(range(NCORES)))
    out = np.concatenate([res.results[c]["OUT"] for c in range(NCORES)], axis=0)
    return out.reshape(B, M, 1).astype(np.int32)
